# revision 27
# baseline (speedup 1.0000x reference)
"""DenseGATLayer Trainium2 Bass kernel, 8-core SPMD.

Sharding: flatten (B=2, N=256) source-node rows into 512 rows, 64 per core.
Cores 0-3 handle batch b=0, cores 4-7 handle b=1. Softmax normalizes over
the j (target node) axis which stays on-core. The small new_node tensor is
AllGathered within each 4-core batch group to build the edge-update C-term.

All heavy matmuls run as fp32 data bitcast to float32r (full PE rate at
moving free-dim >= 256). The xe residual path stays true fp32.
"""
import numpy as np

import concourse.bass as bass
import concourse.mybir as mybir
import concourse.tile as tile
from concourse.bass import ds
from concourse.masks import make_identity
from concourse.vector_clock import ScopedClock

# ---- workaround: walrus CoreV3 setupSyncWait rejects sem waits piggybacked
# on the Tile kernel-tail Drain CTRL instruction. Spill every wait onto its
# own SP nop (executed before the final barrier, so semantics hold).


def _patched_drain_and_barrier(self, tick_clock, wait_clock):
    drain_inst = self.nc.sync.drain()
    wait_clock.add_sem_waits(
        drain_inst.ins, ScopedClock({None: tick_clock.global_clock}))
    si = drain_inst.ins.sync_info
    if si is not None and si.on_wait:
        waits = list(si.on_wait)
        si.on_wait = []
        for k in range(len(waits)):
            nop = self.nc.sync.nop(nofuse=True, hint="drain_wait_split")
            nop.ins.sync_info = mybir.SyncInfo(on_wait=[waits[k]], on_update=[])
    self.nc.all_engine_barrier()
    assert self.sems is not None
    popped = self.nc._tile_sem_poison_stack.pop()
    assert popped is self._sem_poison
    self.nc.clear_and_free_semaphores(list(self.sems.allocated().values()))
    self.nc.all_engine_barrier()


tile.TileContext._drain_and_barrier = _patched_drain_and_barrier


def _split_excess_waits(nc, max_waits=1):
    """Hoist sem waits beyond `max_waits` onto preceding same-engine nops
    (walrus setupSyncWait rejects instructions with too many waits)."""
    for bb in nc.main_func.blocks:
        out = []
        changed = False
        for inst in bb.instructions:
            si = inst.sync_info
            if si is not None and si.on_wait and len(si.on_wait) > max_waits:
                waits = list(si.on_wait)
                for j, w in enumerate(waits[max_waits:]):
                    out.append(mybir.InstNoOp(
                        name=f"{inst.name}-wsplit{j}",
                        sync_info=mybir.SyncInfo(on_wait=[w], on_update=[]),
                        bass_nofuse=True, engine=inst.engine))
                si.on_wait = waits[:max_waits]
                changed = True
            out.append(inst)
        if changed:
            bb.instructions = out

f32 = mybir.dt.float32
f32r = mybir.dt.float32r
bf16 = mybir.dt.bfloat16
AF = mybir.ActivationFunctionType
OP = mybir.AluOpType

B, N, IN, H, D = 2, 256, 256, 8, 64
HD = H * D            # 512
NI = 64               # own source rows per core
KC = 2                # 256 = 2 x 128 chunks (k / f / j axes)
JC = 2
HC = 4                # 512 = 4 x 128 chunks (hd axis)
ALPHA = 0.2
LN_EPS = 1e-5
N_CORES = 8


def _r(ap):
    return ap.bitcast(f32r)


def build():
    nc = bass.Bass(trn_type="TRN2", target_bir_lowering=False, num_devices=N_CORES)

    # ---- per-core external I/O ----
    xe_d = nc.dram_tensor("xe", [NI, N, IN], f32, kind="ExternalInput")
    xb_d = nc.dram_tensor("x_b", [N, IN], f32, kind="ExternalInput")
    xo_d = nc.dram_tensor("x_own", [NI, IN], f32, kind="ExternalInput")
    mb_d = nc.dram_tensor("mask_b", [N, 1], f32, kind="ExternalInput")
    mo_d = nc.dram_tensor("mask_own", [NI, 1], f32, kind="ExternalInput")
    adj_d = nc.dram_tensor("adj", [NI, N], f32, kind="ExternalInput")
    Ws_d = nc.dram_tensor("Ws", [IN, HD], f32, kind="ExternalInput")
    Wt_d = nc.dram_tensor("Wt", [IN, HD], f32, kind="ExternalInput")
    We_d = nc.dram_tensor("We", [IN, HD], f32, kind="ExternalInput")
    attn_d = nc.dram_tensor("attn_w", [H, D], f32, kind="ExternalInput")
    Wn_d = nc.dram_tensor("Wn", [D, IN], f32, kind="ExternalInput")
    bn_d = nc.dram_tensor("bn_b", [1, IN], f32, kind="ExternalInput")
    W1_d = nc.dram_tensor("W1", [IN, IN], f32, kind="ExternalInput")
    W2_d = nc.dram_tensor("W2", [IN, IN], f32, kind="ExternalInput")
    W3_d = nc.dram_tensor("W3", [IN, IN], f32, kind="ExternalInput")
    be_d = nc.dram_tensor("bedge_b", [1, IN], f32, kind="ExternalInput")
    gx_d = nc.dram_tensor("gx", [1, IN], f32, kind="ExternalInput")
    bx_d = nc.dram_tensor("bx", [1, IN], f32, kind="ExternalInput")
    ge_d = nc.dram_tensor("ge", [1, IN], f32, kind="ExternalInput")
    bee_d = nc.dram_tensor("be", [1, IN], f32, kind="ExternalInput")
    on_d = nc.dram_tensor("out_node", [NI, IN], f32, kind="ExternalOutput")
    oe_d = nc.dram_tensor("out_edge", [NI, N, IN], f32, kind="ExternalOutput")

    with tile.TileContext(nc) as tc:
        _body(nc, tc, locals())
    _split_excess_waits(nc)
    return nc


def _body(nc, tc, t):
    xe_d, xb_d, xo_d, mb_d, mo_d, adj_d = (
        t["xe_d"], t["xb_d"], t["xo_d"], t["mb_d"], t["mo_d"], t["adj_d"])
    Ws_d, Wt_d, We_d, attn_d, Wn_d, bn_d = (
        t["Ws_d"], t["Wt_d"], t["We_d"], t["attn_d"], t["Wn_d"], t["bn_d"])
    W1_d, W2_d, W3_d, be_d = t["W1_d"], t["W2_d"], t["W3_d"], t["be_d"]
    gx_d, bx_d, ge_d, bee_d = t["gx_d"], t["bx_d"], t["ge_d"], t["bee_d"]
    on_d, oe_d = t["on_d"], t["oe_d"]

    import contextlib
    ctx = contextlib.ExitStack()
    with ctx:
        const = ctx.enter_context(tc.tile_pool(name="const", bufs=1))
        big = ctx.enter_context(tc.tile_pool(name="big", bufs=1))
        dram = ctx.enter_context(tc.tile_pool(name="dram", bufs=1, space="DRAM"))

        # ---------------- constants & weights ----------------
        I128 = const.tile([128, 128], f32)
        make_identity(nc, I128)
        ones1 = const.tile([1, 128], f32)
        nc.vector.memset(ones1, 1.0)
        eps128 = const.tile([128, 1], f32)
        nc.vector.memset(eps128, LN_EPS)
        eps64 = const.tile([64, 1], f32)
        nc.vector.memset(eps64, LN_EPS)

        s_ctx = contextlib.ExitStack()
        setup0 = s_ctx.enter_context(tc.tile_pool(name="setup0", bufs=2))

        def load_w(name, dram_t, kdim, fdim):
            # stage fp32 from HBM, then round on-chip into a float32r tile so
            # the BIR verifier sees a legal fp32r producer.
            stg = setup0.tile([128, kdim // 128, fdim], f32, tag="wstg")
            nc.sync.dma_start(out=stg, in_=dram_t.rearrange("(c p) h -> p c h", p=128))
            tl = const.tile([128, kdim // 128, fdim], f32r, tag=name)
            nc.scalar.copy(out=tl, in_=stg)
            return tl

        Ws_sb = load_w("Ws_sb", Ws_d, IN, HD)
        Wt_sb = load_w("Wt_sb", Wt_d, IN, HD)
        We_sb = load_w("We_sb", We_d, IN, HD)
        W1_sb = load_w("W1_sb", W1_d, IN, IN)
        W2_sb = load_w("W2_sb", W2_d, IN, IN)
        W3_sb = load_w("W3_sb", W3_d, IN, IN)
        Wn_stg = setup0.tile([D, IN], f32, tag="wstg2")
        nc.sync.dma_start(out=Wn_stg, in_=Wn_d[:, :])
        Wn_sb = const.tile([D, IN], f32r)
        nc.scalar.copy(out=Wn_sb, in_=Wn_stg)
        I128r = const.tile([128, 128], f32r)
        nc.scalar.copy(out=I128r, in_=I128)
        bn_sb = const.tile([1, IN], f32)
        nc.sync.dma_start(out=bn_sb, in_=bn_d[:, :])
        bedge_sb = const.tile([1, IN], f32)
        nc.sync.dma_start(out=bedge_sb, in_=be_d[:, :])
        gx_sb = const.tile([1, IN], f32)
        nc.sync.dma_start(out=gx_sb, in_=gx_d[:, :])
        bx_sb = const.tile([1, IN], f32)
        nc.sync.dma_start(out=bx_sb, in_=bx_d[:, :])
        ge_sb = const.tile([1, IN], f32)
        nc.sync.dma_start(out=ge_sb, in_=ge_d[:, :])
        be_sb = const.tile([1, IN], f32)
        nc.sync.dma_start(out=be_sb, in_=bee_d[:, :])

        # attention vector as block-diagonal (hd-chunk, head) stationary
        attn_stg = setup0.tile([128, HC, H], f32, tag="attn_stg")
        nc.vector.memset(attn_stg, 0.0)
        for h in range(H):
            nc.sync.dma_start(
                out=attn_stg[(h % 2) * 64:(h % 2) * 64 + 64, h // 2, h:h + 1],
                in_=attn_d[h:h + 1, :].rearrange("o d -> d o"))
        attn_bd = const.tile([128, HC, H], f32r)
        nc.vector.tensor_copy(out=attn_bd, in_=attn_stg)

        # ---------------- node features ----------------
        setup = s_ctx.enter_context(tc.tile_pool(name="setup", bufs=1))
        xn_sb = const.tile([128, JC, IN], f32)
        nc.sync.dma_start(out=xn_sb, in_=xb_d.rearrange("(c p) k -> p c k", p=128))
        mask_sb = setup.tile([128, JC, 1], f32)
        nc.sync.dma_start(out=mask_sb, in_=mb_d.rearrange("(c p) o -> p c o", p=128))
        for jc in range(JC):
            nc.vector.tensor_scalar_mul(
                out=xn_sb[:, jc, :], in0=xn_sb[:, jc, :], scalar1=mask_sb[:, jc, :])
        xn_own = const.tile([NI, IN], f32)
        nc.sync.dma_start(out=xn_own, in_=xo_d[:, :])
        mo_sb = setup.tile([NI, 1], f32)
        nc.sync.dma_start(out=mo_sb, in_=mo_d[:, :])
        nc.vector.tensor_scalar_mul(out=xn_own, in0=xn_own, scalar1=mo_sb)

        # ln(adj) computed once, staged to DRAM; re-fetched per-i as a
        # single-partition row for the K=1 rank-1 matmul.
        adj_sb = setup.tile([NI, N], f32)
        nc.sync.dma_start(out=adj_sb, in_=adj_d[:, :])
        nc.scalar.activation(out=adj_sb, in_=adj_sb, func=AF.Ln)
        lnadj_dr = dram.tile([NI, N], f32)
        nc.sync.dma_start(out=lnadj_dr, in_=adj_sb)

        with tc.tile_pool(name="psA", bufs=2, space="PSUM") as psA:
            # xn transpose -> xnT (k, j)
            ps = psA.tile([128, KC, JC, 128], f32, tag="psA")
            for jc in range(JC):
                for kc in range(KC):
                    nc.tensor.transpose(ps[:, kc, jc, :], xn_sb[:, jc, ds(kc * 128, 128)], I128)
            xnT = const.tile([128, KC, N], f32r)
            for kc in range(KC):
                nc.scalar.copy(out=xnT[:, kc, :], in_=ps[:, kc, :, :])

            ps2 = psA.tile([128, KC, NI], f32, tag="psA")
            for kc in range(KC):
                nc.tensor.transpose(ps2[:, kc, :], xn_own[:, ds(kc * 128, 128)],
                                    I128[0:NI, 0:NI])
            xn_ownT = const.tile([128, KC, NI], f32r)
            for kc in range(KC):
                nc.scalar.copy(out=xn_ownT[:, kc, :], in_=ps2[:, kc, :])

            # h_t natural (j, hd) in bf16 for the einsum
            ps3 = psA.tile([128, JC, HD], f32, tag="psA")
            for jc in range(JC):
                for kc in range(KC):
                    nc.tensor.matmul(
                        ps3[:, jc, :], xnT[:, kc, ds(jc * 128, 128)],
                        Wt_sb[:, kc, :], start=(kc == 0), stop=(kc == KC - 1))
            ht_nat = const.tile([128, JC, HD], bf16)
            for jc in range(JC):
                nc.vector.tensor_copy(out=ht_nat[:, jc, :], in_=ps3[:, jc, :])

            # h_t transposed (hd, j) fp32 for attention input
            ps4 = psA.tile([128, HC, N], f32, tag="psA")
            for hc in range(HC):
                for kc in range(KC):
                    nc.tensor.matmul(
                        ps4[:, hc, :], Wt_sb[:, kc, ds(hc * 128, 128)],
                        xnT[:, kc, :], start=(kc == 0), stop=(kc == KC - 1))
            htT = const.tile([128, HC, N], f32)
            for hc in range(HC):
                nc.scalar.copy(out=htT[:, hc, :], in_=ps4[:, hc, :])

            # h_s for own rows -> transposed (hd, i)
            ps5 = psA.tile([NI, HD], f32, tag="psA")
            for kc in range(KC):
                nc.tensor.matmul(ps5, xn_ownT[:, kc, :], Ws_sb[:, kc, :],
                                 start=(kc == 0), stop=(kc == KC - 1))
            hs_nat = setup.tile([NI, HD], f32)
            nc.vector.tensor_copy(out=hs_nat, in_=ps5)
            ps6 = psA.tile([128, HC, NI], f32, tag="psA")
            for hc in range(HC):
                nc.tensor.transpose(ps6[:, hc, :], hs_nat[:, ds(hc * 128, 128)],
                                    I128[0:NI, 0:NI])
            hsT = const.tile([128, HC, NI], f32)
            for hc in range(HC):
                nc.vector.tensor_copy(out=hsT[:, hc, :], in_=ps6[:, hc, :])

            # broadcast LN affine params
            def bcast(row, parts):
                pb = psA.tile([128, IN], f32, tag="psA")
                nc.tensor.matmul(pb[:parts, :], ones1[:, :parts], row[:, :],
                                 start=True, stop=True)
                out = const.tile([parts, IN], f32, tag=f"bc{row.tensor.name}")
                nc.vector.tensor_copy(out=out, in_=pb[:parts, :])
                return out

            gx_bc = bcast(gx_sb, 64)
            bx_bc = bcast(bx_sb, 64)
            ge_bc = bcast(ge_sb, 128)
            be_bc = bcast(be_sb, 128)

        s_ctx.close()  # staging pools freed before the big residents

        # resident big tensors
        xeT_all = big.tile([128, KC, NI, N], f32r)      # 128 KB / partition
        scT_all = big.tile([128, JC, NI, H], bf16)

        # ---------------- phase 1: attention ----------------
        with tc.tile_pool(name="p1sb", bufs=2) as p1sb, \
             tc.tile_pool(name="ps_xeT", bufs=2, space="PSUM") as ps_xeT, \
             tc.tile_pool(name="ps_he", bufs=2, space="PSUM") as ps_he, \
             tc.tile_pool(name="ps_sm", bufs=2, space="PSUM") as ps_sm:
            for i in range(NI):
                xe_t = p1sb.tile([128, JC, IN], f32, tag="xe_t")
                nc.sync.dma_start(out=xe_t, in_=xe_d[i].rearrange("(c p) k -> p c k", p=128))
                pT = ps_xeT.tile([128, KC, JC, 128], f32, tag="pT")
                for jc in range(JC):
                    for kc in range(KC):
                        nc.tensor.transpose(pT[:, kc, jc, :], xe_t[:, jc, ds(kc * 128, 128)], I128)
                for kc in range(KC):
                    nc.scalar.copy(out=xeT_all[:, kc, i, :], in_=pT[:, kc, :, :])

                phe = ps_he.tile([128, HC, N], f32, tag="phe")
                for hc in range(HC):
                    for kc in range(KC):
                        nc.tensor.matmul(
                            phe[:, hc, :], We_sb[:, kc, ds(hc * 128, 128)],
                            xeT_all[:, kc, i, :], start=(kc == 0), stop=(kc == KC - 1))

                lr_t = p1sb.tile([128, HC, N], f32, tag="lr_t")
                for hc in range(HC):
                    nc.vector.scalar_tensor_tensor(
                        out=lr_t[:, hc, :], in0=phe[:, hc, :],
                        scalar=hsT[:, hc, i:i + 1], in1=htT[:, hc, :],
                        op0=OP.add, op1=OP.add)
                ai_t = p1sb.tile([128, HC, N], f32r, tag="ai_t")
                nc.scalar.activation(out=ai_t, in_=lr_t, func=AF.Lrelu, alpha=ALPHA)

                lnrow = p1sb.tile([1, N], f32, tag="lnrow")
                nc.sync.dma_start(out=lnrow, in_=lnadj_dr[i:i + 1, :])
                psc = ps_sm.tile([H, N], f32, tag="psmall")
                for hc in range(HC):
                    nc.tensor.matmul(psc, attn_bd[:, hc, :], ai_t[:, hc, :],
                                     start=(hc == 0), stop=False)
                nc.tensor.matmul(psc, ones1[:, :H], lnrow[:, :],
                                 start=False, stop=True)

                sc_t = p1sb.tile([H, N], f32, tag="sc_t")
                den = p1sb.tile([H, 1], f32, tag="den")
                nc.scalar.activation(out=sc_t, in_=psc, func=AF.Exp, accum_out=den)
                nc.vector.tensor_scalar(out=den, in0=den, scalar1=1e-6, scalar2=None,
                                        op0=OP.add)
                nc.vector.reciprocal(out=den, in_=den)
                nc.vector.tensor_scalar_mul(out=sc_t, in0=sc_t, scalar1=den)
                pst = ps_sm.tile([128, JC, H], f32, tag="psmall")
                for jc in range(JC):
                    nc.tensor.transpose(pst[:, jc, :], sc_t[:, ds(jc * 128, 128)],
                                        I128[0:H, 0:H])
                nc.vector.tensor_copy(out=scT_all[:, :, i, :], in_=pst)

        # ---------------- phase 1.5: node update + allgather ----------------
        p15k = ctx.enter_context(tc.tile_pool(name="p15k", bufs=1))
        with tc.tile_pool(name="p15", bufs=1) as p15, \
             tc.tile_pool(name="psB", bufs=2, space="PSUM") as psB:
            pnn = psB.tile([D, NI], f32, tag="psB")
            first = True
            for h in range(H):
                for jc in range(JC):
                    nc.tensor.matmul(pnn, ht_nat[:, jc, ds(h * D, D)],
                                     scT_all[:, jc, :, h],
                                     start=first, stop=(h == H - 1 and jc == JC - 1))
                    first = False
            nnT_pre = p15.tile([D, NI], f32r)
            nc.vector.tensor_copy(out=nnT_pre, in_=pnn)

            pn2 = psB.tile([NI, IN], f32, tag="psB")
            nc.tensor.matmul(pn2, nnT_pre, Wn_sb, start=True, stop=False)
            nc.tensor.matmul(pn2, ones1[:, :NI], bn_sb, start=False, stop=True)
            new_node = p15.tile([NI, IN], f32)
            nc.vector.tensor_copy(out=new_node, in_=pn2)

            # out_node = LN(xn_own + new_node)
            xpn = p15.tile([NI, IN], f32)
            nc.vector.tensor_tensor(out=xpn, in0=xn_own, in1=new_node, op=OP.add)
            st6 = p15.tile([NI, 6], f32)
            nc.vector.bn_stats(out=st6, in_=xpn)
            mv = p15.tile([NI, 2], f32)
            nc.vector.bn_aggr(out=mv, in_=st6)
            stdn = p15.tile([NI, 1], f32)
            nc.scalar.activation(out=stdn, in_=mv[:, 1:2], func=AF.Sqrt, bias=eps64)
            nc.vector.reciprocal(out=stdn, in_=stdn)
            t_on = p15.tile([NI, IN], f32)
            nc.vector.tensor_scalar(out=t_on, in0=xpn, scalar1=mv[:, 0:1],
                                    scalar2=stdn, op0=OP.subtract, op1=OP.mult)
            tg_on = p15.tile([NI, IN], f32)
            nc.vector.scalar_tensor_tensor(out=tg_on, in0=t_on, scalar=1.0,
                                           in1=gx_bc, op0=OP.mult, op1=OP.mult)
            o_on = p15.tile([NI, IN], f32)
            nc.gpsimd.tensor_tensor(out=o_on, in0=tg_on, in1=bx_bc, op=OP.add)
            nc.sync.dma_start(out=on_d[:, :], in_=o_on)

            # nnT_own (f, i)
            pt = psB.tile([128, KC, NI], f32, tag="psB")
            for kc in range(KC):
                nc.tensor.transpose(pt[:, kc, :], new_node[:, ds(kc * 128, 128)],
                                    I128[0:NI, 0:NI])
            nnT_own = p15.tile([128, KC, NI], f32r)
            for kc in range(KC):
                nc.vector.tensor_copy(out=nnT_own[:, kc, :], in_=pt[:, kc, :])

            # allgather new_node within the batch group
            nn_own_dr = dram.tile([NI, IN], f32)
            nn_all_dr = dram.tile([N, IN], f32)
            nc.sync.dma_start(out=nn_own_dr, in_=new_node)
            nc.gpsimd.collective_compute(
                "AllGather", OP.bypass,
                replica_groups=[[0, 1, 2, 3], [4, 5, 6, 7]],
                ins=[nn_own_dr[:].opt()], outs=[nn_all_dr[:].opt()])
            nn_all_sb = p15.tile([128, JC, IN], f32)
            nc.sync.dma_start(out=nn_all_sb,
                              in_=nn_all_dr[:].rearrange("(c p) f -> p c f", p=128))
            pt2 = psB.tile([128, KC, JC, 128], f32, tag="psB")
            for jc in range(JC):
                for kc in range(KC):
                    nc.tensor.transpose(pt2[:, kc, jc, :],
                                        nn_all_sb[:, jc, ds(kc * 128, 128)], I128)
            nnT_all = p15.tile([128, KC, N], f32r)
            for kc in range(KC):
                nc.scalar.copy(out=nnT_all[:, kc, :], in_=pt2[:, kc, :, :])

            # C^T = W2^T @ nnT_all  (f', j)
            pc = psB.tile([128, KC, N], f32, tag="psB")
            for fc in range(KC):
                for kc in range(KC):
                    nc.tensor.matmul(pc[:, fc, :], W2_sb[:, kc, ds(fc * 128, 128)],
                                     nnT_all[:, kc, :],
                                     start=(kc == 0), stop=(kc == KC - 1))
            C_T = p15k.tile([128, KC, N], f32r)
            for fc in range(KC):
                nc.vector.tensor_copy(out=C_T[:, fc, :], in_=pc[:, fc, :])

            # Abias^T = W1^T @ nnT_own + bedge  (f', i)
            pa = psB.tile([128, KC, NI], f32, tag="psB")
            for fc in range(KC):
                for kc in range(KC):
                    nc.tensor.matmul(pa[:, fc, :], W1_sb[:, kc, ds(fc * 128, 128)],
                                     nnT_own[:, kc, :], start=(kc == 0), stop=False)
                nc.tensor.matmul(pa[:, fc, :], bedge_sb[:, ds(fc * 128, 128)],
                                 ones1[:, :NI], start=False, stop=True)
            AbT = p15k.tile([128, KC, NI], f32)
            for fc in range(KC):
                nc.vector.tensor_copy(out=AbT[:, fc, :], in_=pa[:, fc, :])

        # ---------------- phase 2: edge update ----------------
        with tc.tile_pool(name="p2sb", bufs=2) as p2sb, \
             tc.tile_pool(name="ps_E", bufs=2, space="PSUM") as ps_E, \
             tc.tile_pool(name="ps_En", bufs=2, space="PSUM") as ps_En:
            if True:
                for i in range(NI):
                    pE = ps_E.tile([128, KC, N], f32, tag="pE")
                    for fc in range(KC):
                        for kc in range(KC):
                            nc.tensor.matmul(
                                pE[:, fc, :], W3_sb[:, kc, ds(fc * 128, 128)],
                                xeT_all[:, kc, i, :], start=(kc == 0), stop=False)
                        nc.tensor.matmul(pE[:, fc, :], I128r, C_T[:, fc, :],
                                         start=False, stop=True)
                    ET = p2sb.tile([128, KC, N], f32, tag="ET")
                    for fc in range(KC):
                        nc.vector.scalar_tensor_tensor(
                            out=ET[:, fc, :], in0=pE[:, fc, :],
                            scalar=AbT[:, fc, i:i + 1],
                            in1=xeT_all[:, fc, i, :].bitcast(f32),
                            op0=OP.add, op1=OP.add)
                    pEn = ps_En.tile([128, JC, KC, 128], f32, tag="pEn")
                    for jc in range(JC):
                        for fc in range(KC):
                            nc.tensor.transpose(pEn[:, jc, fc, :],
                                                ET[:, fc, ds(jc * 128, 128)], I128)
                    st = p2sb.tile([128, JC, 6], f32, tag="st")
                    mv2 = p2sb.tile([128, JC, 2], f32, tag="mv2")
                    for jc in range(JC):
                        nc.vector.bn_stats(
                            out=st[:, jc, :],
                            in_=pEn[:, jc, :, :].rearrange("p a b -> p (a b)"))
                        nc.vector.bn_aggr(out=mv2[:, jc, :], in_=st[:, jc, :])
                    std2 = p2sb.tile([128, JC, 1], f32, tag="std2")
                    nc.scalar.activation(out=std2[:, :, 0], in_=mv2[:, :, 1],
                                         func=AF.Sqrt, bias=eps128)
                    nc.vector.reciprocal(out=std2[:, :, 0], in_=std2[:, :, 0])
                    t2 = p2sb.tile([128, JC, N], f32, tag="t2")
                    for jc in range(JC):
                        nc.vector.tensor_scalar(
                            out=t2[:, jc, :], in0=pEn[:, jc, :, :],
                            scalar1=mv2[:, jc, 0:1], scalar2=std2[:, jc, :],
                            op0=OP.subtract, op1=OP.mult)
                    tg2 = p2sb.tile([128, JC, N], f32, tag="tg2")
                    of = p2sb.tile([128, JC, N], f32, tag="of")
                    for jc in range(JC):
                        nc.gpsimd.tensor_tensor(out=tg2[:, jc, :], in0=t2[:, jc, :],
                                                in1=ge_bc, op=OP.mult)
                        nc.gpsimd.tensor_tensor(out=of[:, jc, :], in0=tg2[:, jc, :],
                                                in1=be_bc, op=OP.add)
                    nc.sync.dma_start(
                        out=oe_d[i].rearrange("(c p) f -> p c f", p=128), in_=of)


_NC_CACHE = None


def _get_nc():
    global _NC_CACHE
    if _NC_CACHE is None:
        _NC_CACHE = build()
    return _NC_CACHE


def kernel(emb_node, emb_edge, adj_mtx, node_mask, Ws, Wt, We, attn,
           Wn, bn, Wedge, bedge, gamma_x, beta_x, gamma_e, beta_e):
    from concourse.bass_utils import run_bass_kernel_spmd

    emb_node = np.asarray(emb_node, np.float32)
    emb_edge = np.asarray(emb_edge, np.float32)
    adj_mtx = np.asarray(adj_mtx, np.float32)
    node_mask = np.asarray(node_mask, np.float32)
    Ws = np.asarray(Ws, np.float32)
    Wt = np.asarray(Wt, np.float32)
    We = np.asarray(We, np.float32)
    attn = np.asarray(attn, np.float32).reshape(H, D)
    Wn = np.asarray(Wn, np.float32)
    bn = np.asarray(bn, np.float32).reshape(1, IN)
    Wedge = np.asarray(Wedge, np.float32)
    bedge = np.asarray(bedge, np.float32).reshape(1, IN)
    gamma_x = np.asarray(gamma_x, np.float32).reshape(1, IN)
    beta_x = np.asarray(beta_x, np.float32).reshape(1, IN)
    gamma_e = np.asarray(gamma_e, np.float32).reshape(1, IN)
    beta_e = np.asarray(beta_e, np.float32).reshape(1, IN)
    W1, W2, W3 = Wedge[:IN], Wedge[IN:2 * IN], Wedge[2 * IN:]

    shared = dict(Ws=Ws, Wt=Wt, We=We, attn_w=attn, Wn=Wn, bn_b=bn,
                  W1=np.ascontiguousarray(W1), W2=np.ascontiguousarray(W2),
                  W3=np.ascontiguousarray(W3), bedge_b=bedge,
                  gx=gamma_x, bx=beta_x, ge=gamma_e, be=beta_e)
    in_maps = []
    for c in range(N_CORES):
        b, i0 = c // 4, (c % 4) * NI
        m = dict(shared)
        m["xe"] = np.ascontiguousarray(emb_edge[b, i0:i0 + NI])
        m["x_b"] = np.ascontiguousarray(emb_node[b])
        m["x_own"] = np.ascontiguousarray(emb_node[b, i0:i0 + NI])
        m["mask_b"] = np.ascontiguousarray(node_mask[b].reshape(N, 1))
        m["mask_own"] = np.ascontiguousarray(node_mask[b, i0:i0 + NI].reshape(NI, 1))
        m["adj"] = np.ascontiguousarray(
            adj_mtx[b, i0:i0 + NI, :, 0].reshape(1, NI * N))
        in_maps.append(m)

    res = run_bass_kernel_spmd(_get_nc(), in_maps, core_ids=list(range(N_CORES)))
    global _LAST_RESULTS
    _LAST_RESULTS = res
    out_node = np.empty((B, N, IN), np.float32)
    out_edge = np.empty((B, N, N, IN), np.float32)
    for c in range(N_CORES):
        b, i0 = c // 4, (c % 4) * NI
        out_node[b, i0:i0 + NI] = res.results[c]["out_node"]
        out_edge[b, i0:i0 + NI] = res.results[c]["out_edge"]
    return out_node, out_edge
